# revision 11
# baseline (speedup 1.0000x reference)
"""Local (causal, windowed) attention block on 8 trn2 NeuronCores.

Sharding: sequence-parallel. 8 shards = batch(2) x seq-quarter(4); each core
computes 512 output tokens and needs a 256-token K/V halo on the left.

All matmul operands are bf16 (psum f32). x arrives host-transposed
[feat, tok] so Q/K/scores/attn-out chain without device transposes.
Weights/x load as one big tile each via two parallel HWDGE queues (sync +
scalar) -- per-instruction DMA issue on one queue was a 44us serial cost.

Attention is tiled by 128-query blocks: each (head, qb) takes 3 key-chunks
[far|diag|mid] of 128 keys; mid is always fully inside the window so only
far/diag need a (constant, core-independent) triangle mask -- one in-place
tensor_tensor over [128, 512] per (pair, qb), alternating DVE/GpSimd. exp is
ONE strided activation per (pair, qb) covering both heads' 3 blocks. Halo
out-of-range keys are handled for free: the V ones-column (which generates
softmax denominators via an extra matmul row) is DMA'd as a per-core
valid-flag, zero for zero-padded halo tokens, so pad keys drop out of the
denominator while their V=0 kills the numerator. Denominators gather into an
[8, 512] tile per 4-pair group: one batched reciprocal + cast, then
per-(pair,head) broadcast matmuls against a selector matrix (K=8) produce
the [64, 512] 1/den tiles. bv and bo fold host-side into bo_eff = bv@wo + bo,
added via a ones-row matmul in phase E.

HW quirks: custom-DVE reciprocal must read SBUF at base partition 0, each
psum bank holds exactly one matmul accumulation group (disjoint column
regions within a bank share one start/stop group), and SB+SB tensor_tensor
operands must share a base partition.
"""

import numpy as np
import ml_dtypes

import concourse.bass as bass  # noqa: F401
import concourse.mybir as mybir
import concourse.tile as tile
from concourse import bacc
from concourse.bass_utils import run_bass_kernel_spmd

B, S, D = 2, 2048, 1024
H, DH = 16, 64
WIN = 256
TOK, HALO = 512, 256
XT = TOK + HALO  # 768
F32 = mybir.dt.float32
BF16 = mybir.dt.bfloat16

_cache = {}


def build_nc():
    nc = bacc.Bacc(None, target_bir_lowering=False)
    xhT_d = nc.declare_dram_parameter("xhT", [D, XT], BF16, isOutput=False)
    mall_d = nc.declare_dram_parameter("mall", [128, 512], BF16, isOutput=False)
    wq_d = nc.declare_dram_parameter("wq", [D, D], BF16, isOutput=False)
    wk_d = nc.declare_dram_parameter("wk", [D, D], BF16, isOutput=False)
    wv_d = nc.declare_dram_parameter("wv", [D, D], BF16, isOutput=False)
    wo_d = nc.declare_dram_parameter("wo", [D, D], BF16, isOutput=False)
    bq_d = nc.declare_dram_parameter("bq", [D], F32, isOutput=False)  # pre-scaled 1/8
    bk_d = nc.declare_dram_parameter("bk", [D], F32, isOutput=False)
    valid6_d = nc.declare_dram_parameter("valid6", [128, 6, 16], BF16, isOutput=False)
    onesr_d = nc.declare_dram_parameter("onesr", [1, 128], BF16, isOutput=False)
    bob16_d = nc.declare_dram_parameter("bob16", [1, D], BF16, isOutput=False)
    out_d = nc.declare_dram_parameter("out", [TOK, D], F32, isOutput=True)

    Exp = mybir.ActivationFunctionType.Exp
    Ident = mybir.ActivationFunctionType.Identity

    with tile.TileContext(nc) as tc:
        with (
            tc.tile_pool(name="const", bufs=1) as const,
            tc.tile_pool(name="persist", bufs=1) as persist,
        ):
            # big operand tiles; DMAs split per-weight across the two HWDGE
            # queues (sync + scalar) so x/wq stream in parallel
            xT8 = persist.tile([128, 8, XT], BF16, name="xT8")
            wq8 = persist.tile([128, 8, D], BF16, name="wq8")
            wk8 = persist.tile([128, 8, D], BF16, name="wk8")
            wv8 = persist.tile([128, 8, D], BF16, name="wv8")
            wo8 = persist.tile([128, 8, D], BF16, name="wo8")

            QT = [persist.tile([128, TOK], BF16, name=f"QT{i}") for i in range(8)]
            KT = [persist.tile([128, XT], BF16, name=f"KT{i}") for i in range(8)]
            Vg = persist.tile([128, 6, 16 * 65], BF16, name="Vg")
            AO = [persist.tile([128, TOK], BF16, name=f"AO{i}") for i in range(8)]

            wqr = wq_d[:, :].rearrange("(c p) m -> p c m", p=128)
            xTr = xhT_d[:, :].rearrange("(c p) m -> p c m", p=128)
            wkr = wk_d[:, :].rearrange("(c p) m -> p c m", p=128)
            wvr = wv_d[:, :].rearrange("(c p) m -> p c m", p=128)
            wor = wo_d[:, :].rearrange("(c p) m -> p c m", p=128)
            nc.sync.dma_start(out=xT8[:, 0:4, :], in_=xTr[:, 0:4, :])
            nc.sync.dma_start(out=wq8[:, 0:4, :], in_=wqr[:, 0:4, :])
            nc.sync.dma_start(out=wq8[:, 4:8, :], in_=wqr[:, 4:8, :])
            nc.sync.dma_start(out=xT8[:, 4:8, :], in_=xTr[:, 4:8, :])
            nc.sync.dma_start(out=wk8[:, 0:4, :], in_=wkr[:, 0:4, :])
            nc.sync.dma_start(out=wk8[:, 4:8, :], in_=wkr[:, 4:8, :])
            nc.sync.dma_start(out=wv8[:, 0:4, :], in_=wvr[:, 0:4, :])
            nc.sync.dma_start(out=wv8[:, 4:8, :], in_=wvr[:, 4:8, :])

            # per-outcol-chunk bias columns: [:, 0:8]=bq/8, [:, 8:16]=bk
            bqk = const.tile([128, 16], F32)
            nc.scalar.dma_start(out=bqk[:, 0:8], in_=bq_d[:].rearrange("(c p) -> p c", p=128))
            nc.scalar.dma_start(out=bqk[:, 8:16], in_=bk_d[:].rearrange("(c p) -> p c", p=128))
            mall = const.tile([128, 512], BF16, name="mall")
            nc.scalar.dma_start(out=mall[:], in_=mall_d[:, :])
            onesr = const.tile([1, 128], BF16, name="onesr")
            nc.scalar.dma_start(out=onesr[:], in_=onesr_d[:, :])
            bob16 = const.tile([1, D], BF16, name="bob16")
            nc.scalar.dma_start(out=bob16[:], in_=bob16_d[:, :])
            # V ones-column = per-core valid flags (0 for zero-padded halo
            # tokens: they then drop out of the softmax denominator)
            nc.scalar.dma_start(
                out=Vg[:].rearrange("p t (h d) -> p t h d", d=65)[:, :, :, 64:65],
                in_=valid6_d[:, :, :].rearrange("p t (h o) -> p t h o", o=1))
            nc.sync.dma_start(out=wo8[:, 0:4, :], in_=wor[:, 0:4, :])
            nc.sync.dma_start(out=wo8[:, 4:8, :], in_=wor[:, 4:8, :])

            # ---- Phase B: QT = (wq.T @ xT)/8 + bq/8 ; KT = wk.T @ xT + bk ----
            with tc.tile_pool(name="qpsum", bufs=4, space="PSUM") as qpsum:
                for oc in range(8):
                    ps = qpsum.tile([128, TOK], F32, tag="pp")
                    for kc in range(8):
                        nc.tensor.matmul(
                            ps[:],
                            lhsT=wq8[:, kc, oc * 128:(oc + 1) * 128],
                            rhs=xT8[:, kc, HALO:XT],
                            start=(kc == 0), stop=(kc == 7),
                        )
                    # QT pre-scaled by 1/8 (bias arrives pre-scaled from host)
                    nc.scalar.activation(QT[oc][:], ps[:], Ident,
                                         bias=bqk[:, oc:oc + 1], scale=0.125)
                for oc in range(8):
                    for hf in range(2):
                        ps = qpsum.tile([128, 384], F32, tag="pp", padded_shape=[128, 512])
                        for kc in range(8):
                            nc.tensor.matmul(
                                ps[:],
                                lhsT=wk8[:, kc, oc * 128:(oc + 1) * 128],
                                rhs=xT8[:, kc, hf * 384:(hf + 1) * 384],
                                start=(kc == 0), stop=(kc == 7),
                            )
                        nc.scalar.activation(KT[oc][:, hf * 384:(hf + 1) * 384],
                                             ps[:], Ident,
                                             bias=bqk[:, 8 + oc:9 + oc], scale=1.0)

                # ---- Phase C: V (natural layout, no bias; bv folds into E) ----
                for tt in range(6):
                    for hf in range(2):
                        ps = qpsum.tile([128, 512], F32, tag="pp")
                        for kc in range(8):
                            nc.tensor.matmul(
                                ps[:],
                                lhsT=xT8[:, kc, tt * 128:(tt + 1) * 128],
                                rhs=wv8[:, kc, hf * 512:(hf + 1) * 512],
                                start=(kc == 0), stop=(kc == 7),
                            )
                        dst = Vg[:, tt, hf * 520:(hf + 1) * 520].rearrange(
                            "p (h d) -> p h d", d=65)[:, :, 0:64]
                        # split psum evacuation across ACT and DVE
                        if tt % 2 == 0:
                            nc.scalar.copy(dst, ps[:].rearrange("p (h d) -> p h d", d=64))
                        else:
                            nc.vector.tensor_copy(dst, ps[:].rearrange("p (h d) -> p h d", d=64))

            # ---- Phase D: attention, 128-query-block tiled ----
            # per (pair, qb): psum [128k, 1024] = h0 bank0 / h1 bank1, blocks
            # far@0 diag@128 mid@256 (q columns); es [128, 768] =
            # [far_h0|far_h1|diag_h0|diag_h1|mid_h0|mid_h1]
            with (
                tc.tile_pool(name="spsum", bufs=2, space="PSUM") as spsum,
                tc.tile_pool(name="opsum", bufs=2, space="PSUM") as opsum,
                tc.tile_pool(name="es", bufs=10) as es_pool,
                tc.tile_pool(name="aou", bufs=6) as aou_pool,
                tc.tile_pool(name="den", bufs=2) as den_pool,
            ):
                es_all = [[None] * 4 for _ in range(8)]
                po_all = [None] * 8
                aop_all = [None] * 8

                def emit_scores_qb(p, qb):
                    g = p
                    es = es_pool.tile([128, 768], BF16, tag="es")
                    ps = spsum.tile([128, 1024], F32, tag="sp")
                    # blocks: far (keys qb*128), diag (keys (qb+2)*128),
                    # mid (keys (qb+1)*128) -- KT columns are halo-offset
                    for bi, kc in ((0, qb), (1, qb + 2), (2, qb + 1)):
                        for h2 in (0, 1):
                            ho = h2 * 64
                            nc.tensor.matmul(
                                ps[:, 512 * h2 + bi * 128: 512 * h2 + (bi + 1) * 128],
                                lhsT=KT[g][ho:ho + 64, kc * 128:(kc + 1) * 128],
                                rhs=QT[g][ho:ho + 64, qb * 128:(qb + 1) * 128],
                                start=(bi == 0), stop=(bi == 2),
                            )
                    # one exp over both heads' 3 blocks
                    nc.scalar.activation(
                        es[:].rearrange("p (b h q) -> p b h q", b=3, h=2),
                        ps[:].rearrange("p (h b q) -> p b h q", h=2, b=4)[:, 0:3],
                        Exp)
                    # triangle masks on far+diag only (mid always in-window)
                    with nc.allow_low_precision(reason="bf16 es mask"):
                        nc.gpsimd.tensor_mul(es[:, 0:512], es[:, 0:512], mall[:, 0:512])
                    es_all[p][qb] = es

                def emit_av_qb(p, qb):
                    es = es_all[p][qb]
                    if qb == 0:
                        po_all[p] = opsum.tile([65, 2 * TOK], F32, tag="op",
                                               name=f"po{p}")
                    po = po_all[p]
                    for h2 in (0, 1):
                        h = 2 * p + h2
                        # far/diag/mid -> V key-chunks qb, qb+2, qb+1
                        for bi, kc in ((0, qb), (1, qb + 2), (2, qb + 1)):
                            nc.tensor.matmul(
                                po[:, h2 * TOK + qb * 128: h2 * TOK + (qb + 1) * 128],
                                lhsT=Vg[:, kc, h * 65:(h + 1) * 65],
                                rhs=es[:, bi * 256 + h2 * 128: bi * 256 + (h2 + 1) * 128],
                                start=(qb == 0 and bi == 0), stop=(qb == 3 and bi == 2),
                            )

                def emit_den(p):
                    # single-copy po evacuation (den row spans both banks);
                    # reciprocal reads base-0 SBUF (HW quirk)
                    po = po_all[p]
                    aop = aou_pool.tile([64, 2 * TOK], F32, tag="ao")
                    den2 = den_pool.tile([1, 2 * TOK], F32, tag="dn")
                    nc.scalar.copy(den2[:], po[64:65, :])
                    nc.vector.tensor_copy(aop[:], po[0:64, :])
                    rsc = den_pool.tile([1, 2 * TOK], F32, tag="rs")
                    nc.vector.reciprocal_approx_fast(rsc[:], den2[:])
                    rcb = den_pool.tile([1, 2 * TOK], BF16, tag="rb")
                    with nc.allow_low_precision(reason="bf16 1/den"):
                        nc.vector.tensor_copy(rcb[:], rsc[:])
                    pbt = opsum.tile([65, 2 * TOK], F32, tag="op", name=f"pb{p}")
                    for h2 in (0, 1):
                        nc.tensor.matmul(
                            pbt[0:64, h2 * TOK:(h2 + 1) * TOK], lhsT=onesr[:, 0:64],
                            rhs=rcb[:, h2 * TOK:(h2 + 1) * TOK],
                            start=True, stop=True,
                        )
                        with nc.allow_low_precision(reason="bf16 attn output"):
                            nc.vector.tensor_mul(AO[p][64 * h2:64 * h2 + 64, :],
                                                 pbt[0:64, h2 * TOK:(h2 + 1) * TOK],
                                                 aop[:, h2 * TOK:(h2 + 1) * TOK])

                for p in range(9):
                    for qb in range(4):
                        if p < 8:
                            emit_scores_qb(p, qb)
                        if p >= 1:
                            emit_av_qb(p - 1, qb)
                    if p >= 1:
                        emit_den(p - 1)

            # ---- Phase E: out = AO.T @ wo + bo_eff ----
            with (
                tc.tile_pool(name="fpsum", bufs=5, space="PSUM") as fpsum,
                tc.tile_pool(name="oout", bufs=3) as oout,
            ):
                for tt in range(4):
                    ot = oout.tile([128, D], F32, tag="oo")
                    for hf in range(2):
                        ps = fpsum.tile([128, 512], F32, tag="fp")
                        nc.tensor.matmul(
                            ps[:], lhsT=onesr[:],
                            rhs=bob16[:, hf * 512:(hf + 1) * 512],
                            start=True, stop=False,
                        )
                        for kc in range(8):
                            nc.tensor.matmul(
                                ps[:],
                                lhsT=AO[kc][:, tt * 128:(tt + 1) * 128],
                                rhs=wo8[:, kc, hf * 512:(hf + 1) * 512],
                                start=False, stop=(kc == 7),
                            )
                        if hf == 0:
                            nc.scalar.copy(ot[:, hf * 512:(hf + 1) * 512], ps[:])
                        else:
                            nc.vector.tensor_copy(ot[:, hf * 512:(hf + 1) * 512], ps[:])
                        nc.sync.dma_start(
                            out=out_d[tt * 128:(tt + 1) * 128, hf * 512:(hf + 1) * 512],
                            in_=ot[:, hf * 512:(hf + 1) * 512])

    nc.compile()
    return nc


def kernel(x, wq, bq, wk, bk, wv, bv, wo, bo):
    bf = ml_dtypes.bfloat16
    x = np.asarray(x, np.float32)
    wq16 = np.ascontiguousarray(np.asarray(wq, np.float32).astype(bf))
    wk16 = np.ascontiguousarray(np.asarray(wk, np.float32).astype(bf))
    wv16 = np.ascontiguousarray(np.asarray(wv, np.float32).astype(bf))
    wo32 = np.asarray(wo, np.float32)
    wo16 = np.ascontiguousarray(wo32.astype(bf))
    bq8 = np.ascontiguousarray(np.asarray(bq, np.float32) * 0.125)
    bk = np.ascontiguousarray(np.asarray(bk, np.float32))
    # fold bv through wo (softmax rows sum to 1): out += bv @ wo + bo
    bo_eff = (np.asarray(bv, np.float32) @ wo32 + np.asarray(bo, np.float32)).astype(np.float32)
    bob16 = np.ascontiguousarray(bo_eff.reshape(1, D).astype(bf))
    onesr = np.ones((1, 128), bf)

    if "nc" not in _cache:
        _cache["nc"] = build_nc()
        # triangle masks: block order [far_h0|far_h1|diag_h0|diag_h1];
        # far valid iff q_idx <= k_idx (q-k = j-i+256 <= 256), diag valid
        # iff q_idx >= k_idx (q-k = j-i >= 0); both include the diagonal
        i = np.arange(128)[:, None]
        j = np.arange(128)[None, :]
        triL = (j <= i).astype(np.float32)
        triU = (j >= i).astype(np.float32)
        mall = np.concatenate([triL, triL, triU, triU], axis=1)
        _cache["mall"] = np.ascontiguousarray(mall.astype(bf))
        # per-core V valid flags: [128, 6, 16]; zero where the halo token is
        # zero-padding (core c==0: tokens 0..255 i.e. chunks 0,1)
        v_ok = np.ones((128, 6, 16), np.float32)
        v_pad = v_ok.copy()
        v_pad[:, 0:2, :] = 0.0
        _cache["valid_ok"] = np.ascontiguousarray(v_ok.astype(bf))
        _cache["valid_pad"] = np.ascontiguousarray(v_pad.astype(bf))
    nc = _cache["nc"]

    in_maps = []
    for core in range(8):
        b, c = divmod(core, 4)
        start = c * TOK
        xh = np.zeros((XT, D), np.float32)
        lo = max(0, start - HALO)
        xh[HALO - (start - lo):] = x[b, lo:start + TOK]
        xhT = np.ascontiguousarray(xh.T.astype(bf))
        in_maps.append({
            "xhT": xhT,
            "wq": wq16, "wk": wk16, "wv": wv16, "wo": wo16,
            "bq": bq8, "bk": bk,
            "valid6": _cache["valid_pad"] if c == 0 else _cache["valid_ok"],
            "mall": _cache["mall"],
            "bob16": bob16, "onesr": onesr,
        })
    _cache["last_in_maps"] = in_maps
    res = run_bass_kernel_spmd(nc, in_maps, list(range(8)))
    out = np.empty((B, S, D), np.float32)
    for core in range(8):
        b, c = divmod(core, 4)
        out[b, c * TOK:(c + 1) * TOK] = res.results[core]["out"]
    return out


# revision 13
# speedup vs baseline: 1.0560x; 1.0560x over previous
"""Local (causal, windowed) attention block on 8 trn2 NeuronCores.

Sharding: sequence-parallel. 8 shards = batch(2) x seq-quarter(4); each core
computes 512 output tokens and needs a 256-token K/V halo on the left.

All matmul operands are bf16 (psum f32). x arrives host-transposed
[feat, tok] so Q/K/scores/attn-out chain without device transposes.
Weights/x load as one big tile each via two parallel HWDGE queues (sync +
scalar) -- per-instruction DMA issue on one queue was a 44us serial cost.

Attention is tiled by 128-query blocks: each (head, qb) takes 3 key-chunks
[far|diag|mid] of 128 keys; mid is always fully inside the window so only
far/diag need a (constant, core-independent) triangle mask -- one in-place
tensor_tensor over [128, 512] per (pair, qb), alternating DVE/GpSimd. exp is
ONE strided activation per (pair, qb) covering both heads' 3 blocks. Halo
out-of-range keys are handled for free: the V ones-column (which generates
softmax denominators via an extra matmul row) is DMA'd as a per-core
valid-flag, zero for zero-padded halo tokens, so pad keys drop out of the
denominator while their V=0 kills the numerator. Denominators gather into an
[8, 512] tile per 4-pair group: one batched reciprocal + cast, then
per-(pair,head) broadcast matmuls against a selector matrix (K=8) produce
the [64, 512] 1/den tiles. bv and bo fold host-side into bo_eff = bv@wo + bo,
added via a ones-row matmul in phase E.

HW quirks: custom-DVE reciprocal must read SBUF at base partition 0, each
psum bank holds exactly one matmul accumulation group (disjoint column
regions within a bank share one start/stop group), and SB+SB tensor_tensor
operands must share a base partition.
"""

import numpy as np
import ml_dtypes

import concourse.bass as bass  # noqa: F401
import concourse.mybir as mybir
import concourse.tile as tile
from concourse import bacc
from concourse.bass_utils import run_bass_kernel_spmd

B, S, D = 2, 2048, 1024
H, DH = 16, 64
WIN = 256
TOK, HALO = 512, 256
XT = TOK + HALO  # 768
F32 = mybir.dt.float32
F32R = mybir.dt.float32r
BF16 = mybir.dt.bfloat16

_cache = {}


def build_nc():
    nc = bacc.Bacc(None, target_bir_lowering=False)
    xhT_d = nc.declare_dram_parameter("xhT", [D, XT], BF16, isOutput=False)
    mall_d = nc.declare_dram_parameter("mall", [128, 512], BF16, isOutput=False)
    wq_d = nc.declare_dram_parameter("wq", [D, D], BF16, isOutput=False)
    wk_d = nc.declare_dram_parameter("wk", [D, D], BF16, isOutput=False)
    wv_d = nc.declare_dram_parameter("wv", [D, D], BF16, isOutput=False)
    wo_d = nc.declare_dram_parameter("wo", [D, D], BF16, isOutput=False)
    bq_d = nc.declare_dram_parameter("bq", [D], F32, isOutput=False)  # pre-scaled 1/8
    bk_d = nc.declare_dram_parameter("bk", [D], F32, isOutput=False)
    valid6_d = nc.declare_dram_parameter("valid6", [128, 6, 16], BF16, isOutput=False)
    onesr_d = nc.declare_dram_parameter("onesr", [1, 128], BF16, isOutput=False)
    ones64f_d = nc.declare_dram_parameter("ones64f", [1, 64], F32, isOutput=False)
    bob16_d = nc.declare_dram_parameter("bob16", [1, D], BF16, isOutput=False)
    out_d = nc.declare_dram_parameter("out", [TOK, D], F32, isOutput=True)

    Exp = mybir.ActivationFunctionType.Exp
    Ident = mybir.ActivationFunctionType.Identity

    with tile.TileContext(nc) as tc:
        with (
            tc.tile_pool(name="const", bufs=1) as const,
            tc.tile_pool(name="persist", bufs=1) as persist,
        ):
            # big operand tiles; DMAs split per-weight across the two HWDGE
            # queues (sync + scalar) so x/wq stream in parallel
            xT8 = persist.tile([128, 8, XT], BF16, name="xT8")
            wq8 = persist.tile([128, 8, D], BF16, name="wq8")
            wk8 = persist.tile([128, 8, D], BF16, name="wk8")
            wv8 = persist.tile([128, 8, D], BF16, name="wv8")
            wo8 = persist.tile([128, 8, D], BF16, name="wo8")

            QT = [persist.tile([128, TOK], BF16, name=f"QT{i}") for i in range(8)]
            KT = [persist.tile([128, XT], BF16, name=f"KT{i}") for i in range(8)]
            Vg = persist.tile([128, 6, 16 * 65], BF16, name="Vg")
            AO = [persist.tile([128, TOK], BF16, name=f"AO{i}") for i in range(8)]

            wqr = wq_d[:, :].rearrange("(c p) m -> p c m", p=128)
            xTr = xhT_d[:, :].rearrange("(c p) m -> p c m", p=128)
            wkr = wk_d[:, :].rearrange("(c p) m -> p c m", p=128)
            wvr = wv_d[:, :].rearrange("(c p) m -> p c m", p=128)
            wor = wo_d[:, :].rearrange("(c p) m -> p c m", p=128)
            nc.sync.dma_start(out=xT8[:, 0:4, :], in_=xTr[:, 0:4, :])
            nc.sync.dma_start(out=wq8[:, 0:4, :], in_=wqr[:, 0:4, :])
            nc.sync.dma_start(out=wq8[:, 4:8, :], in_=wqr[:, 4:8, :])
            nc.sync.dma_start(out=xT8[:, 4:8, :], in_=xTr[:, 4:8, :])
            nc.sync.dma_start(out=wk8[:, 0:4, :], in_=wkr[:, 0:4, :])
            nc.sync.dma_start(out=wk8[:, 4:8, :], in_=wkr[:, 4:8, :])
            nc.sync.dma_start(out=wv8[:, 0:4, :], in_=wvr[:, 0:4, :])
            nc.sync.dma_start(out=wv8[:, 4:8, :], in_=wvr[:, 4:8, :])

            # per-outcol-chunk bias columns: [:, 0:8]=bq/8, [:, 8:16]=bk
            bqk = const.tile([128, 16], F32)
            nc.scalar.dma_start(out=bqk[:, 0:8], in_=bq_d[:].rearrange("(c p) -> p c", p=128))
            nc.scalar.dma_start(out=bqk[:, 8:16], in_=bk_d[:].rearrange("(c p) -> p c", p=128))
            mall = const.tile([128, 512], BF16, name="mall")
            nc.scalar.dma_start(out=mall[:], in_=mall_d[:, :])
            onesr = const.tile([1, 128], BF16, name="onesr")
            nc.scalar.dma_start(out=onesr[:], in_=onesr_d[:, :])
            ones64f = const.tile([1, 64], F32, name="ones64f")
            nc.scalar.dma_start(out=ones64f[:], in_=ones64f_d[:, :])
            bob16 = const.tile([1, D], BF16, name="bob16")
            nc.scalar.dma_start(out=bob16[:], in_=bob16_d[:, :])
            # V ones-column = per-core valid flags (0 for zero-padded halo
            # tokens: they then drop out of the softmax denominator)
            nc.scalar.dma_start(
                out=Vg[:].rearrange("p t (h d) -> p t h d", d=65)[:, :, :, 64:65],
                in_=valid6_d[:, :, :].rearrange("p t (h o) -> p t h o", o=1))
            nc.sync.dma_start(out=wo8[:, 0:4, :], in_=wor[:, 0:4, :])
            nc.sync.dma_start(out=wo8[:, 4:8, :], in_=wor[:, 4:8, :])

            # ---- Phase B: QT = (wq.T @ xT)/8 + bq/8 ; KT = wk.T @ xT + bk ----
            with tc.tile_pool(name="qpsum", bufs=4, space="PSUM") as qpsum:
                for oc in range(8):
                    ps = qpsum.tile([128, TOK], F32, tag="pp")
                    for kc in range(8):
                        nc.tensor.matmul(
                            ps[:],
                            lhsT=wq8[:, kc, oc * 128:(oc + 1) * 128],
                            rhs=xT8[:, kc, HALO:XT],
                            start=(kc == 0), stop=(kc == 7),
                        )
                    # QT pre-scaled by 1/8 (bias arrives pre-scaled from host)
                    nc.scalar.activation(QT[oc][:], ps[:], Ident,
                                         bias=bqk[:, oc:oc + 1], scale=0.125)
                for oc in range(8):
                    for hf in range(2):
                        ps = qpsum.tile([128, 384], F32, tag="pp", padded_shape=[128, 512])
                        for kc in range(8):
                            nc.tensor.matmul(
                                ps[:],
                                lhsT=wk8[:, kc, oc * 128:(oc + 1) * 128],
                                rhs=xT8[:, kc, hf * 384:(hf + 1) * 384],
                                start=(kc == 0), stop=(kc == 7),
                            )
                        nc.scalar.activation(KT[oc][:, hf * 384:(hf + 1) * 384],
                                             ps[:], Ident,
                                             bias=bqk[:, 8 + oc:9 + oc], scale=1.0)

                # ---- Phase C: V (natural layout, no bias; bv folds into E) ----
                for tt in range(6):
                    for hf in range(2):
                        ps = qpsum.tile([128, 512], F32, tag="pp")
                        for kc in range(8):
                            nc.tensor.matmul(
                                ps[:],
                                lhsT=xT8[:, kc, tt * 128:(tt + 1) * 128],
                                rhs=wv8[:, kc, hf * 512:(hf + 1) * 512],
                                start=(kc == 0), stop=(kc == 7),
                            )
                        dst = Vg[:, tt, hf * 520:(hf + 1) * 520].rearrange(
                            "p (h d) -> p h d", d=65)[:, :, 0:64]
                        # split psum evacuation across ACT and DVE
                        if tt % 2 == 0:
                            nc.scalar.copy(dst, ps[:].rearrange("p (h d) -> p h d", d=64))
                        else:
                            nc.vector.tensor_copy(dst, ps[:].rearrange("p (h d) -> p h d", d=64))

            # ---- Phase D: attention, 128-query-block tiled ----
            # per (pair, qb): psum [128k, 1024] = h0 bank0 / h1 bank1, blocks
            # far@0 diag@128 mid@256 (q columns); es [128, 768] =
            # [far_h0|far_h1|diag_h0|diag_h1|mid_h0|mid_h1]
            with (
                tc.tile_pool(name="spsum", bufs=2, space="PSUM") as spsum,
                tc.tile_pool(name="opsum", bufs=2, space="PSUM") as opsum,
                tc.tile_pool(name="es", bufs=10) as es_pool,
                tc.tile_pool(name="aou", bufs=6) as aou_pool,
                tc.tile_pool(name="den", bufs=2) as den_pool,
            ):
                es_all = [[None] * 4 for _ in range(8)]
                po_all = [None] * 8
                aop_all = [None] * 8

                def emit_scores_qb(p, qb):
                    g = p
                    es = es_pool.tile([128, 768], BF16, tag="es")
                    ps = spsum.tile([128, 1024], F32, tag="sp")
                    # blocks: far (keys qb*128), diag (keys (qb+2)*128),
                    # mid (keys (qb+1)*128) -- KT columns are halo-offset
                    for bi, kc in ((0, qb), (1, qb + 2), (2, qb + 1)):
                        for h2 in (0, 1):
                            ho = h2 * 64
                            nc.tensor.matmul(
                                ps[:, 512 * h2 + bi * 128: 512 * h2 + (bi + 1) * 128],
                                lhsT=KT[g][ho:ho + 64, kc * 128:(kc + 1) * 128],
                                rhs=QT[g][ho:ho + 64, qb * 128:(qb + 1) * 128],
                                start=(bi == 0), stop=(bi == 2),
                            )
                    # one exp over both heads' 3 blocks
                    nc.scalar.activation(
                        es[:].rearrange("p (b h q) -> p b h q", b=3, h=2),
                        ps[:].rearrange("p (h b q) -> p b h q", h=2, b=4)[:, 0:3],
                        Exp)
                    # triangle masks on far+diag only (mid always in-window)
                    with nc.allow_low_precision(reason="bf16 es mask"):
                        eng = nc.vector if qb == 1 else nc.gpsimd
                        eng.tensor_mul(es[:, 0:512], es[:, 0:512], mall[:, 0:512])
                    es_all[p][qb] = es

                def emit_av_qb(p, qb):
                    es = es_all[p][qb]
                    if qb == 0:
                        po_all[p] = opsum.tile([65, 2 * TOK], F32, tag="op",
                                               name=f"po{p}")
                    po = po_all[p]
                    for h2 in (0, 1):
                        h = 2 * p + h2
                        # far/diag/mid -> V key-chunks qb, qb+2, qb+1
                        for bi, kc in ((0, qb), (1, qb + 2), (2, qb + 1)):
                            nc.tensor.matmul(
                                po[:, h2 * TOK + qb * 128: h2 * TOK + (qb + 1) * 128],
                                lhsT=Vg[:, kc, h * 65:(h + 1) * 65],
                                rhs=es[:, bi * 256 + h2 * 128: bi * 256 + (h2 + 1) * 128],
                                start=(qb == 0 and bi == 0), stop=(qb == 3 and bi == 2),
                            )

                def emit_den(p):
                    # single-copy po evacuation (den row spans both banks);
                    # reciprocal reads base-0 SBUF (HW quirk)
                    po = po_all[p]
                    aop = aou_pool.tile([64, 2 * TOK], F32, tag="ao")
                    den2 = den_pool.tile([1, 2 * TOK], F32, tag="dn")
                    nc.scalar.copy(den2[:], po[64:65, :])
                    nc.vector.tensor_copy(aop[:], po[0:64, :])
                    rsc = den_pool.tile([1, 2 * TOK], F32, tag="rs")
                    nc.vector.reciprocal_approx_fast(rsc[:], den2[:])
                    rcb = den_pool.tile([1, 2 * TOK], BF16, tag="rb")
                    with nc.allow_low_precision(reason="bf16 1/den"):
                        nc.vector.tensor_copy(rcb[:], rsc[:])
                    pbt = opsum.tile([65, 2 * TOK], F32, tag="op", name=f"pb{p}")
                    for h2 in (0, 1):
                        nc.tensor.matmul(
                            pbt[0:64, h2 * TOK:(h2 + 1) * TOK], lhsT=onesr[:, 0:64],
                            rhs=rcb[:, h2 * TOK:(h2 + 1) * TOK],
                            start=True, stop=True,
                        )
                        with nc.allow_low_precision(reason="bf16 attn output"):
                            nc.vector.tensor_mul(AO[p][64 * h2:64 * h2 + 64, :],
                                                 pbt[0:64, h2 * TOK:(h2 + 1) * TOK],
                                                 aop[:, h2 * TOK:(h2 + 1) * TOK])

                for p in range(9):
                    for qb in range(4):
                        if p < 8:
                            emit_scores_qb(p, qb)
                        if p >= 1:
                            emit_av_qb(p - 1, qb)
                    if p >= 1:
                        emit_den(p - 1)

            # ---- Phase E: out = AO.T @ wo + bo_eff ----
            with (
                tc.tile_pool(name="fpsum", bufs=5, space="PSUM") as fpsum,
                tc.tile_pool(name="oout", bufs=3) as oout,
            ):
                for tt in range(4):
                    ot = oout.tile([128, D], F32, tag="oo")
                    for hf in range(2):
                        ps = fpsum.tile([128, 512], F32, tag="fp")
                        nc.tensor.matmul(
                            ps[:], lhsT=onesr[:],
                            rhs=bob16[:, hf * 512:(hf + 1) * 512],
                            start=True, stop=False,
                        )
                        for kc in range(8):
                            nc.tensor.matmul(
                                ps[:],
                                lhsT=AO[kc][:, tt * 128:(tt + 1) * 128],
                                rhs=wo8[:, kc, hf * 512:(hf + 1) * 512],
                                start=False, stop=(kc == 7),
                            )
                        if hf == 0:
                            nc.scalar.copy(ot[:, hf * 512:(hf + 1) * 512], ps[:])
                        else:
                            nc.vector.tensor_copy(ot[:, hf * 512:(hf + 1) * 512], ps[:])
                        nc.sync.dma_start(
                            out=out_d[tt * 128:(tt + 1) * 128, hf * 512:(hf + 1) * 512],
                            in_=ot[:, hf * 512:(hf + 1) * 512])

    nc.compile()
    return nc


def kernel(x, wq, bq, wk, bk, wv, bv, wo, bo):
    bf = ml_dtypes.bfloat16
    x = np.asarray(x, np.float32)
    wq16 = np.ascontiguousarray(np.asarray(wq, np.float32).astype(bf))
    wk16 = np.ascontiguousarray(np.asarray(wk, np.float32).astype(bf))
    wv16 = np.ascontiguousarray(np.asarray(wv, np.float32).astype(bf))
    wo32 = np.asarray(wo, np.float32)
    wo16 = np.ascontiguousarray(wo32.astype(bf))
    bq8 = np.ascontiguousarray(np.asarray(bq, np.float32) * 0.125)
    bk = np.ascontiguousarray(np.asarray(bk, np.float32))
    # fold bv through wo (softmax rows sum to 1): out += bv @ wo + bo
    bo_eff = (np.asarray(bv, np.float32) @ wo32 + np.asarray(bo, np.float32)).astype(np.float32)
    bob16 = np.ascontiguousarray(bo_eff.reshape(1, D).astype(bf))
    onesr = np.ones((1, 128), bf)

    if "nc" not in _cache:
        _cache["nc"] = build_nc()
        # triangle masks: block order [far_h0|far_h1|diag_h0|diag_h1];
        # far valid iff q_idx <= k_idx (q-k = j-i+256 <= 256), diag valid
        # iff q_idx >= k_idx (q-k = j-i >= 0); both include the diagonal
        i = np.arange(128)[:, None]
        j = np.arange(128)[None, :]
        triL = (j <= i).astype(np.float32)
        triU = (j >= i).astype(np.float32)
        mall = np.concatenate([triL, triL, triU, triU], axis=1)
        _cache["mall"] = np.ascontiguousarray(mall.astype(bf))
        # per-core V valid flags: [128, 6, 16]; zero where the halo token is
        # zero-padding (core c==0: tokens 0..255 i.e. chunks 0,1)
        v_ok = np.ones((128, 6, 16), np.float32)
        v_pad = v_ok.copy()
        v_pad[:, 0:2, :] = 0.0
        _cache["valid_ok"] = np.ascontiguousarray(v_ok.astype(bf))
        _cache["valid_pad"] = np.ascontiguousarray(v_pad.astype(bf))
    nc = _cache["nc"]

    in_maps = []
    for core in range(8):
        b, c = divmod(core, 4)
        start = c * TOK
        xh = np.zeros((XT, D), np.float32)
        lo = max(0, start - HALO)
        xh[HALO - (start - lo):] = x[b, lo:start + TOK]
        xhT = np.ascontiguousarray(xh.T.astype(bf))
        in_maps.append({
            "xhT": xhT,
            "wq": wq16, "wk": wk16, "wv": wv16, "wo": wo16,
            "bq": bq8, "bk": bk,
            "valid6": _cache["valid_pad"] if c == 0 else _cache["valid_ok"],
            "mall": _cache["mall"],
            "bob16": bob16, "onesr": onesr,
            "ones64f": np.ones((1, 64), np.float32),
        })
    _cache["last_in_maps"] = in_maps
    res = run_bass_kernel_spmd(nc, in_maps, list(range(8)))
    out = np.empty((B, S, D), np.float32)
    for core in range(8):
        b, c = divmod(core, 4)
        out[b, c * TOK:(c + 1) * TOK] = res.results[core]["out"]
    return out


# revision 15
# speedup vs baseline: 1.0577x; 1.0016x over previous
"""Local (causal, windowed) attention block on 8 trn2 NeuronCores.

Sharding: sequence-parallel. 8 shards = batch(2) x seq-quarter(4); each core
computes 512 output tokens and needs a 256-token K/V halo on the left.

All matmul operands are bf16 (psum f32). x arrives host-transposed
[feat, tok] so Q/K/scores/attn-out chain without device transposes.
Weights/x load as one big tile each via two parallel HWDGE queues (sync +
scalar) -- per-instruction DMA issue on one queue was a 44us serial cost.

Attention is tiled by 128-query blocks: each (head, qb) takes 3 key-chunks
[far|diag|mid] of 128 keys; mid is always fully inside the window so only
far/diag need a (constant, core-independent) triangle mask -- one in-place
tensor_tensor over [128, 512] per (pair, qb), alternating DVE/GpSimd. exp is
ONE strided activation per (pair, qb) covering both heads' 3 blocks. Halo
out-of-range keys are handled for free: the V ones-column (which generates
softmax denominators via an extra matmul row) is DMA'd as a per-core
valid-flag, zero for zero-padded halo tokens, so pad keys drop out of the
denominator while their V=0 kills the numerator. Denominators gather into an
[8, 512] tile per 4-pair group: one batched reciprocal + cast, then
per-(pair,head) broadcast matmuls against a selector matrix (K=8) produce
the [64, 512] 1/den tiles. bv and bo fold host-side into bo_eff = bv@wo + bo,
added via a ones-row matmul in phase E.

HW quirks: custom-DVE reciprocal must read SBUF at base partition 0, each
psum bank holds exactly one matmul accumulation group (disjoint column
regions within a bank share one start/stop group), and SB+SB tensor_tensor
operands must share a base partition.
"""

import numpy as np
import ml_dtypes

import concourse.bass as bass  # noqa: F401
import concourse.mybir as mybir
import concourse.tile as tile
from concourse import bacc
from concourse.bass_utils import run_bass_kernel_spmd

B, S, D = 2, 2048, 1024
H, DH = 16, 64
WIN = 256
TOK, HALO = 512, 256
XT = TOK + HALO  # 768
F32 = mybir.dt.float32
F32R = mybir.dt.float32r
BF16 = mybir.dt.bfloat16

_cache = {}


def build_nc():
    nc = bacc.Bacc(None, target_bir_lowering=False)
    xhT_d = nc.declare_dram_parameter("xhT", [D, XT], BF16, isOutput=False)
    mall_d = nc.declare_dram_parameter("mall", [128, 512], BF16, isOutput=False)
    wq_d = nc.declare_dram_parameter("wq", [D, D], BF16, isOutput=False)
    wk_d = nc.declare_dram_parameter("wk", [D, D], BF16, isOutput=False)
    wv_d = nc.declare_dram_parameter("wv", [D, D], BF16, isOutput=False)
    wo_d = nc.declare_dram_parameter("wo", [D, D], BF16, isOutput=False)
    bq_d = nc.declare_dram_parameter("bq", [D], F32, isOutput=False)  # pre-scaled 1/8
    bk_d = nc.declare_dram_parameter("bk", [D], F32, isOutput=False)
    valid6_d = nc.declare_dram_parameter("valid6", [128, 6, 16], BF16, isOutput=False)
    onesr_d = nc.declare_dram_parameter("onesr", [1, 128], BF16, isOutput=False)
    ones64f_d = nc.declare_dram_parameter("ones64f", [1, 64], F32, isOutput=False)
    bob16_d = nc.declare_dram_parameter("bob16", [1, D], BF16, isOutput=False)
    out_d = nc.declare_dram_parameter("out", [TOK, D], F32, isOutput=True)

    Exp = mybir.ActivationFunctionType.Exp
    Ident = mybir.ActivationFunctionType.Identity

    with tile.TileContext(nc) as tc:
        with (
            tc.tile_pool(name="const", bufs=1) as const,
            tc.tile_pool(name="persist", bufs=1) as persist,
        ):
            # big operand tiles; DMAs split per-weight across the two HWDGE
            # queues (sync + scalar) so x/wq stream in parallel
            xT8 = persist.tile([128, 8, XT], BF16, name="xT8")
            wq8 = persist.tile([128, 8, D], BF16, name="wq8")
            wk8 = persist.tile([128, 8, D], BF16, name="wk8")
            wv8 = persist.tile([128, 8, D], BF16, name="wv8")
            wo8 = persist.tile([128, 8, D], BF16, name="wo8")

            QT = [persist.tile([128, TOK], BF16, name=f"QT{i}") for i in range(8)]
            KT = [persist.tile([128, XT], BF16, name=f"KT{i}") for i in range(8)]
            Vg = persist.tile([128, 6, 16 * 65], BF16, name="Vg")
            AO = [persist.tile([128, TOK], BF16, name=f"AO{i}") for i in range(8)]

            wqr = wq_d[:, :].rearrange("(c p) m -> p c m", p=128)
            xTr = xhT_d[:, :].rearrange("(c p) m -> p c m", p=128)
            wkr = wk_d[:, :].rearrange("(c p) m -> p c m", p=128)
            wvr = wv_d[:, :].rearrange("(c p) m -> p c m", p=128)
            wor = wo_d[:, :].rearrange("(c p) m -> p c m", p=128)
            nc.sync.dma_start(out=xT8[:, 0:4, :], in_=xTr[:, 0:4, :])
            nc.sync.dma_start(out=wq8[:, 0:4, :], in_=wqr[:, 0:4, :])
            nc.sync.dma_start(out=wq8[:, 4:8, :], in_=wqr[:, 4:8, :])
            nc.sync.dma_start(out=xT8[:, 4:8, :], in_=xTr[:, 4:8, :])
            nc.sync.dma_start(out=wk8[:, 0:4, :], in_=wkr[:, 0:4, :])
            nc.sync.dma_start(out=wk8[:, 4:8, :], in_=wkr[:, 4:8, :])
            nc.sync.dma_start(out=wv8[:, 0:4, :], in_=wvr[:, 0:4, :])
            nc.sync.dma_start(out=wv8[:, 4:8, :], in_=wvr[:, 4:8, :])

            # per-outcol-chunk bias columns: [:, 0:8]=bq/8, [:, 8:16]=bk
            bqk = const.tile([128, 16], F32)
            nc.scalar.dma_start(out=bqk[:, 0:8], in_=bq_d[:].rearrange("(c p) -> p c", p=128))
            nc.scalar.dma_start(out=bqk[:, 8:16], in_=bk_d[:].rearrange("(c p) -> p c", p=128))
            mall = const.tile([128, 512], BF16, name="mall")
            nc.scalar.dma_start(out=mall[:], in_=mall_d[:, :])
            onesr = const.tile([1, 128], BF16, name="onesr")
            nc.scalar.dma_start(out=onesr[:], in_=onesr_d[:, :])
            ones64f = const.tile([1, 64], F32, name="ones64f")
            nc.scalar.dma_start(out=ones64f[:], in_=ones64f_d[:, :])
            bob16 = const.tile([1, D], BF16, name="bob16")
            nc.scalar.dma_start(out=bob16[:], in_=bob16_d[:, :])
            # V ones-column = per-core valid flags (0 for zero-padded halo
            # tokens: they then drop out of the softmax denominator)
            nc.scalar.dma_start(
                out=Vg[:].rearrange("p t (h d) -> p t h d", d=65)[:, :, :, 64:65],
                in_=valid6_d[:, :, :].rearrange("p t (h o) -> p t h o", o=1))
            nc.sync.dma_start(out=wo8[:, 0:4, :], in_=wor[:, 0:4, :])
            nc.sync.dma_start(out=wo8[:, 4:8, :], in_=wor[:, 4:8, :])

            # ---- Phase B: QT = (wq.T @ xT)/8 + bq/8 ; KT = wk.T @ xT + bk ----
            with tc.tile_pool(name="qpsum", bufs=4, space="PSUM") as qpsum:
                for oc in range(8):
                    ps = qpsum.tile([128, TOK], F32, tag="pp")
                    for kc in range(8):
                        nc.tensor.matmul(
                            ps[:],
                            lhsT=wq8[:, kc, oc * 128:(oc + 1) * 128],
                            rhs=xT8[:, kc, HALO:XT],
                            start=(kc == 0), stop=(kc == 7),
                        )
                    # QT pre-scaled by 1/8 (bias arrives pre-scaled from host)
                    nc.scalar.activation(QT[oc][:], ps[:], Ident,
                                         bias=bqk[:, oc:oc + 1], scale=0.125)
                for oc in range(8):
                    for hf in range(2):
                        ps = qpsum.tile([128, 384], F32, tag="pp", padded_shape=[128, 512])
                        for kc in range(8):
                            nc.tensor.matmul(
                                ps[:],
                                lhsT=wk8[:, kc, oc * 128:(oc + 1) * 128],
                                rhs=xT8[:, kc, hf * 384:(hf + 1) * 384],
                                start=(kc == 0), stop=(kc == 7),
                            )
                        nc.scalar.activation(KT[oc][:, hf * 384:(hf + 1) * 384],
                                             ps[:], Ident,
                                             bias=bqk[:, 8 + oc:9 + oc], scale=1.0)

            # ---- Phase C+D: V projection + attention (qb-tiled) ----
            # per (pair, qb): psum [128k, 1024] = h0 bank0 / h1 bank1, blocks
            # far@0 diag@128 mid@256 (q columns); es [128, 768] =
            # [far_h0|far_h1|diag_h0|diag_h1|mid_h0|mid_h1].
            # pair-0 scores are hoisted before V so the softmax pipeline is
            # primed when AV starts; pb matmuls lag one pair behind their
            # den-chain so the in-order PE queue never waits on DVE.
            with (
                tc.tile_pool(name="spsum", bufs=2, space="PSUM") as spsum,
                tc.tile_pool(name="es", bufs=14) as es_pool,
                tc.tile_pool(name="aou", bufs=6) as aou_pool,
                tc.tile_pool(name="den", bufs=3) as den_pool,
            ):
                es_all = [[None] * 4 for _ in range(8)]
                po_all = [None] * 8
                aop_all = [None] * 8
                rcb_all = [None] * 8

                def emit_scores_qb(p, qb):
                    g = p
                    es = es_pool.tile([128, 768], BF16, tag="es")
                    ps = spsum.tile([128, 1024], F32, tag="sp")
                    # blocks: far (keys qb*128), diag (keys (qb+2)*128),
                    # mid (keys (qb+1)*128) -- KT columns are halo-offset
                    for bi, kc in ((0, qb), (1, qb + 2), (2, qb + 1)):
                        for h2 in (0, 1):
                            ho = h2 * 64
                            nc.tensor.matmul(
                                ps[:, 512 * h2 + bi * 128: 512 * h2 + (bi + 1) * 128],
                                lhsT=KT[g][ho:ho + 64, kc * 128:(kc + 1) * 128],
                                rhs=QT[g][ho:ho + 64, qb * 128:(qb + 1) * 128],
                                start=(bi == 0), stop=(bi == 2),
                            )
                    # one exp over both heads' 3 blocks
                    nc.scalar.activation(
                        es[:].rearrange("p (b h q) -> p b h q", b=3, h=2),
                        ps[:].rearrange("p (h b q) -> p b h q", h=2, b=4)[:, 0:3],
                        Exp)
                    # triangle masks on far+diag only (mid always in-window)
                    with nc.allow_low_precision(reason="bf16 es mask"):
                        eng = nc.vector if qb == 1 else nc.gpsimd
                        eng.tensor_mul(es[:, 0:512], es[:, 0:512], mall[:, 0:512])
                    es_all[p][qb] = es

                def emit_av_qb(p, qb):
                    es = es_all[p][qb]
                    if qb == 0:
                        po_all[p] = opsum.tile([65, 2 * TOK], F32, tag="op",
                                               name=f"po{p}")
                    po = po_all[p]
                    for h2 in (0, 1):
                        h = 2 * p + h2
                        # far/diag/mid -> V key-chunks qb, qb+2, qb+1
                        for bi, kc in ((0, qb), (1, qb + 2), (2, qb + 1)):
                            nc.tensor.matmul(
                                po[:, h2 * TOK + qb * 128: h2 * TOK + (qb + 1) * 128],
                                lhsT=Vg[:, kc, h * 65:(h + 1) * 65],
                                rhs=es[:, bi * 256 + h2 * 128: bi * 256 + (h2 + 1) * 128],
                                start=(qb == 0 and bi == 0), stop=(qb == 3 and bi == 2),
                            )

                def emit_evac(p):
                    # single-copy po evacuation (den row spans both banks);
                    # reciprocal reads base-0 SBUF (HW quirk)
                    po = po_all[p]
                    aop = aou_pool.tile([64, 2 * TOK], F32, tag="ao")
                    den2 = den_pool.tile([1, 2 * TOK], F32, tag="dn")
                    nc.scalar.copy(den2[:], po[64:65, :])
                    nc.vector.tensor_copy(aop[:], po[0:64, :])
                    rsc = den_pool.tile([1, 2 * TOK], F32, tag="rs")
                    nc.vector.reciprocal_approx_fast(rsc[:], den2[:])
                    rcb = den_pool.tile([1, 2 * TOK], BF16, tag="rb")
                    with nc.allow_low_precision(reason="bf16 1/den"):
                        nc.vector.tensor_copy(rcb[:], rsc[:])
                    aop_all[p] = aop
                    rcb_all[p] = rcb

                def emit_norm(p):
                    aop, rcb = aop_all[p], rcb_all[p]
                    pbt = opsum.tile([65, 2 * TOK], F32, tag="op", name=f"pb{p}")
                    for h2 in (0, 1):
                        nc.tensor.matmul(
                            pbt[0:64, h2 * TOK:(h2 + 1) * TOK], lhsT=onesr[:, 0:64],
                            rhs=rcb[:, h2 * TOK:(h2 + 1) * TOK],
                            start=True, stop=True,
                        )
                        with nc.allow_low_precision(reason="bf16 attn output"):
                            nc.vector.tensor_mul(AO[p][64 * h2:64 * h2 + 64, :],
                                                 pbt[0:64, h2 * TOK:(h2 + 1) * TOK],
                                                 aop[:, h2 * TOK:(h2 + 1) * TOK])

                # prime the pipeline: pair-0 scores before the V projection
                for qb in range(4):
                    emit_scores_qb(0, qb)

                # ---- V projection (own 2-bank psum pool; fits beside sp) ----
                with tc.tile_pool(name="vpsum", bufs=2, space="PSUM") as vpsum:
                    for tt in range(6):
                        for hf in range(2):
                            ps = vpsum.tile([128, 512], F32, tag="vp")
                            for kc in range(8):
                                nc.tensor.matmul(
                                    ps[:],
                                    lhsT=xT8[:, kc, tt * 128:(tt + 1) * 128],
                                    rhs=wv8[:, kc, hf * 512:(hf + 1) * 512],
                                    start=(kc == 0), stop=(kc == 7),
                                )
                            dst = Vg[:, tt, hf * 520:(hf + 1) * 520].rearrange(
                                "p (h d) -> p h d", d=65)[:, :, 0:64]
                            # split psum evacuation across ACT and DVE
                            if tt % 2 == 0:
                                nc.scalar.copy(dst, ps[:].rearrange("p (h d) -> p h d", d=64))
                            else:
                                nc.vector.tensor_copy(dst, ps[:].rearrange("p (h d) -> p h d", d=64))

                with tc.tile_pool(name="opsum", bufs=2, space="PSUM") as opsum:
                    for p in range(1, 10):
                        for qb in range(4):
                            if p < 8:
                                emit_scores_qb(p, qb)
                            if p <= 8:
                                emit_av_qb(p - 1, qb)
                        if p <= 8:
                            emit_evac(p - 1)
                        if p >= 2:
                            emit_norm(p - 2)

            # ---- Phase E: out = AO.T @ wo + bo_eff ----
            with (
                tc.tile_pool(name="fpsum", bufs=5, space="PSUM") as fpsum,
                tc.tile_pool(name="oout", bufs=3) as oout,
            ):
                for tt in range(4):
                    ot = oout.tile([128, D], F32, tag="oo")
                    for hf in range(2):
                        ps = fpsum.tile([128, 512], F32, tag="fp")
                        nc.tensor.matmul(
                            ps[:], lhsT=onesr[:],
                            rhs=bob16[:, hf * 512:(hf + 1) * 512],
                            start=True, stop=False,
                        )
                        for kc in range(8):
                            nc.tensor.matmul(
                                ps[:],
                                lhsT=AO[kc][:, tt * 128:(tt + 1) * 128],
                                rhs=wo8[:, kc, hf * 512:(hf + 1) * 512],
                                start=False, stop=(kc == 7),
                            )
                        if hf == 0:
                            nc.scalar.copy(ot[:, hf * 512:(hf + 1) * 512], ps[:])
                        else:
                            nc.vector.tensor_copy(ot[:, hf * 512:(hf + 1) * 512], ps[:])
                        nc.sync.dma_start(
                            out=out_d[tt * 128:(tt + 1) * 128, hf * 512:(hf + 1) * 512],
                            in_=ot[:, hf * 512:(hf + 1) * 512])

    nc.compile()
    return nc


def kernel(x, wq, bq, wk, bk, wv, bv, wo, bo):
    bf = ml_dtypes.bfloat16
    x = np.asarray(x, np.float32)
    wq16 = np.ascontiguousarray(np.asarray(wq, np.float32).astype(bf))
    wk16 = np.ascontiguousarray(np.asarray(wk, np.float32).astype(bf))
    wv16 = np.ascontiguousarray(np.asarray(wv, np.float32).astype(bf))
    wo32 = np.asarray(wo, np.float32)
    wo16 = np.ascontiguousarray(wo32.astype(bf))
    bq8 = np.ascontiguousarray(np.asarray(bq, np.float32) * 0.125)
    bk = np.ascontiguousarray(np.asarray(bk, np.float32))
    # fold bv through wo (softmax rows sum to 1): out += bv @ wo + bo
    bo_eff = (np.asarray(bv, np.float32) @ wo32 + np.asarray(bo, np.float32)).astype(np.float32)
    bob16 = np.ascontiguousarray(bo_eff.reshape(1, D).astype(bf))
    onesr = np.ones((1, 128), bf)

    if "nc" not in _cache:
        _cache["nc"] = build_nc()
        # triangle masks: block order [far_h0|far_h1|diag_h0|diag_h1];
        # far valid iff q_idx <= k_idx (q-k = j-i+256 <= 256), diag valid
        # iff q_idx >= k_idx (q-k = j-i >= 0); both include the diagonal
        i = np.arange(128)[:, None]
        j = np.arange(128)[None, :]
        triL = (j <= i).astype(np.float32)
        triU = (j >= i).astype(np.float32)
        mall = np.concatenate([triL, triL, triU, triU], axis=1)
        _cache["mall"] = np.ascontiguousarray(mall.astype(bf))
        # per-core V valid flags: [128, 6, 16]; zero where the halo token is
        # zero-padding (core c==0: tokens 0..255 i.e. chunks 0,1)
        v_ok = np.ones((128, 6, 16), np.float32)
        v_pad = v_ok.copy()
        v_pad[:, 0:2, :] = 0.0
        _cache["valid_ok"] = np.ascontiguousarray(v_ok.astype(bf))
        _cache["valid_pad"] = np.ascontiguousarray(v_pad.astype(bf))
    nc = _cache["nc"]

    in_maps = []
    for core in range(8):
        b, c = divmod(core, 4)
        start = c * TOK
        xh = np.zeros((XT, D), np.float32)
        lo = max(0, start - HALO)
        xh[HALO - (start - lo):] = x[b, lo:start + TOK]
        xhT = np.ascontiguousarray(xh.T.astype(bf))
        in_maps.append({
            "xhT": xhT,
            "wq": wq16, "wk": wk16, "wv": wv16, "wo": wo16,
            "bq": bq8, "bk": bk,
            "valid6": _cache["valid_pad"] if c == 0 else _cache["valid_ok"],
            "mall": _cache["mall"],
            "bob16": bob16, "onesr": onesr,
            "ones64f": np.ones((1, 64), np.float32),
        })
    _cache["last_in_maps"] = in_maps
    res = run_bass_kernel_spmd(nc, in_maps, list(range(8)))
    out = np.empty((B, S, D), np.float32)
    for core in range(8):
        b, c = divmod(core, 4)
        out[b, c * TOK:(c + 1) * TOK] = res.results[core]["out"]
    return out


# revision 16
# speedup vs baseline: 1.1840x; 1.1194x over previous
"""Local (causal, windowed) attention block on 8 trn2 NeuronCores.

Sharding: sequence-parallel. 8 shards = batch(2) x seq-quarter(4); each core
computes 512 output tokens and needs a 256-token K/V halo on the left.

All matmul operands are bf16 (psum f32). x arrives host-transposed
[feat, tok] so Q/K/scores/attn-out chain without device transposes.
Weights/x load as one big tile each via two parallel HWDGE queues (sync +
scalar) -- per-instruction DMA issue on one queue was a 44us serial cost.

Attention is tiled by 128-query blocks: each (head, qb) takes 3 key-chunks
[far|diag|mid] of 128 keys; mid is always fully inside the window so only
far/diag need a (constant, core-independent) triangle mask -- one in-place
tensor_tensor over [128, 512] per (pair, qb), alternating DVE/GpSimd. exp is
ONE strided activation per (pair, qb) covering both heads' 3 blocks. Halo
out-of-range keys are handled for free: the V ones-column (which generates
softmax denominators via an extra matmul row) is DMA'd as a per-core
valid-flag, zero for zero-padded halo tokens, so pad keys drop out of the
denominator while their V=0 kills the numerator. Denominators gather into an
[8, 512] tile per 4-pair group: one batched reciprocal + cast, then
per-(pair,head) broadcast matmuls against a selector matrix (K=8) produce
the [64, 512] 1/den tiles. bv and bo fold host-side into bo_eff = bv@wo + bo,
added via a ones-row matmul in phase E.

HW quirks: custom-DVE reciprocal must read SBUF at base partition 0, each
psum bank holds exactly one matmul accumulation group (disjoint column
regions within a bank share one start/stop group), and SB+SB tensor_tensor
operands must share a base partition.
"""

import numpy as np
import ml_dtypes

import concourse.bass as bass  # noqa: F401
import concourse.mybir as mybir
import concourse.tile as tile
from concourse import bacc
from concourse.bass_utils import run_bass_kernel_spmd

B, S, D = 2, 2048, 1024
H, DH = 16, 64
WIN = 256
TOK, HALO = 512, 256
XT = TOK + HALO  # 768
F32 = mybir.dt.float32
F32R = mybir.dt.float32r
BF16 = mybir.dt.bfloat16

_cache = {}


def build_nc():
    nc = bacc.Bacc(None, target_bir_lowering=False)
    xhT_d = nc.declare_dram_parameter("xhT", [D, XT], BF16, isOutput=False)
    mall_d = nc.declare_dram_parameter("mall", [128, 512], BF16, isOutput=False)
    wq_d = nc.declare_dram_parameter("wq", [D, D], BF16, isOutput=False)
    wk_d = nc.declare_dram_parameter("wk", [D, D], BF16, isOutput=False)
    wv_d = nc.declare_dram_parameter("wv", [D, D], BF16, isOutput=False)
    wo_d = nc.declare_dram_parameter("wo", [D, D], BF16, isOutput=False)
    bq_d = nc.declare_dram_parameter("bq", [D], F32, isOutput=False)  # pre-scaled 1/8
    bk_d = nc.declare_dram_parameter("bk", [D], F32, isOutput=False)
    valid6_d = nc.declare_dram_parameter("valid6", [128, 6, 16], BF16, isOutput=False)
    onesr_d = nc.declare_dram_parameter("onesr", [1, 128], BF16, isOutput=False)
    ones64f_d = nc.declare_dram_parameter("ones64f", [1, 64], F32, isOutput=False)
    bob16_d = nc.declare_dram_parameter("bob16", [1, D], BF16, isOutput=False)
    out_d = nc.declare_dram_parameter("out", [TOK, D], F32, isOutput=True)

    Exp = mybir.ActivationFunctionType.Exp
    Ident = mybir.ActivationFunctionType.Identity

    with tile.TileContext(nc) as tc:
        with (
            tc.tile_pool(name="const", bufs=1) as const,
            tc.tile_pool(name="persist", bufs=1) as persist,
        ):
            # big operand tiles; DMAs split per-weight across the two HWDGE
            # queues (sync + scalar) so x/wq stream in parallel
            xT8 = persist.tile([128, 8, XT], BF16, name="xT8")
            wq8 = persist.tile([128, 8, D], BF16, name="wq8")
            wk8 = persist.tile([128, 8, D], BF16, name="wk8")
            wv8 = persist.tile([128, 8, D], BF16, name="wv8")
            wo8 = persist.tile([128, 8, D], BF16, name="wo8")

            QT = [persist.tile([128, TOK], BF16, name=f"QT{i}") for i in range(8)]
            KT = [persist.tile([128, XT], BF16, name=f"KT{i}") for i in range(8)]
            Vg = persist.tile([128, 6, 16 * 65], BF16, name="Vg")
            AO = [persist.tile([128, TOK], BF16, name=f"AO{i}") for i in range(8)]

            wqr = wq_d[:, :].rearrange("(c p) m -> p c m", p=128)
            xTr = xhT_d[:, :].rearrange("(c p) m -> p c m", p=128)
            wkr = wk_d[:, :].rearrange("(c p) m -> p c m", p=128)
            wvr = wv_d[:, :].rearrange("(c p) m -> p c m", p=128)
            wor = wo_d[:, :].rearrange("(c p) m -> p c m", p=128)
            nc.sync.dma_start(out=xT8[:, 0:4, :], in_=xTr[:, 0:4, :])
            nc.sync.dma_start(out=wq8[:, 0:4, :], in_=wqr[:, 0:4, :])
            nc.sync.dma_start(out=wq8[:, 4:8, :], in_=wqr[:, 4:8, :])
            nc.sync.dma_start(out=xT8[:, 4:8, :], in_=xTr[:, 4:8, :])
            nc.sync.dma_start(out=wk8[:, 0:4, :], in_=wkr[:, 0:4, :])
            nc.sync.dma_start(out=wk8[:, 4:8, :], in_=wkr[:, 4:8, :])
            nc.sync.dma_start(out=wv8[:, 0:4, :], in_=wvr[:, 0:4, :])
            nc.sync.dma_start(out=wv8[:, 4:8, :], in_=wvr[:, 4:8, :])

            # per-outcol-chunk bias columns: [:, 0:8]=bq/8, [:, 8:16]=bk
            bqk = const.tile([128, 16], F32)
            nc.scalar.dma_start(out=bqk[:, 0:8], in_=bq_d[:].rearrange("(c p) -> p c", p=128))
            nc.scalar.dma_start(out=bqk[:, 8:16], in_=bk_d[:].rearrange("(c p) -> p c", p=128))
            mall = const.tile([128, 512], BF16, name="mall")
            nc.scalar.dma_start(out=mall[:], in_=mall_d[:, :])
            onesr = const.tile([1, 128], BF16, name="onesr")
            nc.scalar.dma_start(out=onesr[:], in_=onesr_d[:, :])
            ones64f = const.tile([1, 64], F32, name="ones64f")
            nc.scalar.dma_start(out=ones64f[:], in_=ones64f_d[:, :])
            bob16 = const.tile([1, D], BF16, name="bob16")
            nc.scalar.dma_start(out=bob16[:], in_=bob16_d[:, :])
            # V ones-column = per-core valid flags (0 for zero-padded halo
            # tokens: they then drop out of the softmax denominator)
            nc.scalar.dma_start(
                out=Vg[:].rearrange("p t (h d) -> p t h d", d=65)[:, :, :, 64:65],
                in_=valid6_d[:, :, :].rearrange("p t (h o) -> p t h o", o=1))
            nc.sync.dma_start(out=wo8[:, 0:4, :], in_=wor[:, 0:4, :])
            nc.sync.dma_start(out=wo8[:, 4:8, :], in_=wor[:, 4:8, :])

            # ---- Phase B: QT = (wq.T @ xT)/8 + bq/8 ; KT = wk.T @ xT + bk ----
            with tc.tile_pool(name="qpsum", bufs=4, space="PSUM") as qpsum:
                for oc in range(8):
                    ps = qpsum.tile([128, TOK], F32, tag="pp")
                    for kc in range(8):
                        nc.tensor.matmul(
                            ps[:],
                            lhsT=wq8[:, kc, oc * 128:(oc + 1) * 128],
                            rhs=xT8[:, kc, HALO:XT],
                            start=(kc == 0), stop=(kc == 7),
                        )
                    # QT pre-scaled by 1/8 (bias arrives pre-scaled from host)
                    nc.scalar.activation(QT[oc][:], ps[:], Ident,
                                         bias=bqk[:, oc:oc + 1], scale=0.125)
                for oc in range(8):
                    for hf in range(2):
                        ps = qpsum.tile([128, 384], F32, tag="pp", padded_shape=[128, 512])
                        for kc in range(8):
                            nc.tensor.matmul(
                                ps[:],
                                lhsT=wk8[:, kc, oc * 128:(oc + 1) * 128],
                                rhs=xT8[:, kc, hf * 384:(hf + 1) * 384],
                                start=(kc == 0), stop=(kc == 7),
                            )
                        nc.scalar.activation(KT[oc][:, hf * 384:(hf + 1) * 384],
                                             ps[:], Ident,
                                             bias=bqk[:, 8 + oc:9 + oc], scale=1.0)

            # ---- Phase C+D: V projection + attention (qb-tiled) ----
            # per (pair, qb): psum [128k, 1024] = h0 bank0 / h1 bank1, blocks
            # far@0 diag@128 mid@256 (q columns); es [128, 768] =
            # [far_h0|far_h1|diag_h0|diag_h1|mid_h0|mid_h1].
            # pair-0 scores are hoisted before V so the softmax pipeline is
            # primed when AV starts; pb matmuls lag one pair behind their
            # den-chain so the in-order PE queue never waits on DVE.
            with (
                tc.tile_pool(name="spsum", bufs=2, space="PSUM") as spsum,
                tc.tile_pool(name="es", bufs=14) as es_pool,
                tc.tile_pool(name="aou", bufs=6) as aou_pool,
                tc.tile_pool(name="den", bufs=3) as den_pool,
            ):
                es_all = [[None] * 4 for _ in range(8)]
                po_all = [None] * 8
                aop_all = [None] * 8
                rcb_all = [None] * 8

                def emit_scores_qb(p, qb):
                    g = p
                    es = es_pool.tile([128, 768], BF16, tag="es")
                    ps = spsum.tile([128, 1024], F32, tag="sp")
                    # blocks: far (keys qb*128), diag (keys (qb+2)*128),
                    # mid (keys (qb+1)*128) -- KT columns are halo-offset
                    for bi, kc in ((0, qb), (1, qb + 2), (2, qb + 1)):
                        for h2 in (0, 1):
                            ho = h2 * 64
                            nc.tensor.matmul(
                                ps[:, 512 * h2 + bi * 128: 512 * h2 + (bi + 1) * 128],
                                lhsT=KT[g][ho:ho + 64, kc * 128:(kc + 1) * 128],
                                rhs=QT[g][ho:ho + 64, qb * 128:(qb + 1) * 128],
                                start=(bi == 0), stop=(bi == 2),
                            )
                    # one exp over both heads' 3 blocks
                    nc.scalar.activation(
                        es[:].rearrange("p (b h q) -> p b h q", b=3, h=2),
                        ps[:].rearrange("p (h b q) -> p b h q", h=2, b=4)[:, 0:3],
                        Exp)
                    # triangle masks on far+diag only (mid always in-window)
                    with nc.allow_low_precision(reason="bf16 es mask"):
                        eng = nc.vector if qb == 1 else nc.gpsimd
                        eng.tensor_mul(es[:, 0:512], es[:, 0:512], mall[:, 0:512])
                    es_all[p][qb] = es

                def emit_av_qb(p, qb):
                    es = es_all[p][qb]
                    if qb == 0:
                        po_all[p] = opsum.tile([65, 2 * TOK], F32, tag="op",
                                               name=f"po{p}")
                    po = po_all[p]
                    for h2 in (0, 1):
                        h = 2 * p + h2
                        # far/diag/mid -> V key-chunks qb, qb+2, qb+1
                        for bi, kc in ((0, qb), (1, qb + 2), (2, qb + 1)):
                            nc.tensor.matmul(
                                po[:, h2 * TOK + qb * 128: h2 * TOK + (qb + 1) * 128],
                                lhsT=Vg[:, kc, h * 65:(h + 1) * 65],
                                rhs=es[:, bi * 256 + h2 * 128: bi * 256 + (h2 + 1) * 128],
                                start=(qb == 0 and bi == 0), stop=(qb == 3 and bi == 2),
                            )

                def emit_evac(p):
                    # single-copy po evacuation (den row spans both banks);
                    # reciprocal reads base-0 SBUF (HW quirk)
                    po = po_all[p]
                    aop = aou_pool.tile([64, 2 * TOK], F32, tag="ao")
                    den2 = den_pool.tile([1, 2 * TOK], F32, tag="dn")
                    nc.scalar.copy(den2[:], po[64:65, :])
                    nc.vector.tensor_copy(aop[:], po[0:64, :])
                    rsc = den_pool.tile([1, 2 * TOK], F32, tag="rs")
                    nc.vector.reciprocal_approx_fast(rsc[:], den2[:])
                    rcb = den_pool.tile([1, 2 * TOK], BF16, tag="rb")
                    with nc.allow_low_precision(reason="bf16 1/den"):
                        nc.vector.tensor_copy(rcb[:], rsc[:])
                    aop_all[p] = aop
                    rcb_all[p] = rcb

                def emit_norm(p):
                    aop, rcb = aop_all[p], rcb_all[p]
                    pbt = opsum.tile([65, 2 * TOK], F32, tag="op", name=f"pb{p}")
                    for h2 in (0, 1):
                        nc.tensor.matmul(
                            pbt[0:64, h2 * TOK:(h2 + 1) * TOK], lhsT=onesr[:, 0:64],
                            rhs=rcb[:, h2 * TOK:(h2 + 1) * TOK],
                            start=True, stop=True,
                        )
                        with nc.allow_low_precision(reason="bf16 attn output"):
                            nc.vector.tensor_mul(AO[p][64 * h2:64 * h2 + 64, :],
                                                 pbt[0:64, h2 * TOK:(h2 + 1) * TOK],
                                                 aop[:, h2 * TOK:(h2 + 1) * TOK])

                def emit_v(tt):
                    # V psum shares the scores pool (bank budget: sp4 + op4)
                    for hf in range(2):
                        ps = spsum.tile([128, 512], F32, tag="sp",
                                        padded_shape=[128, 1024], name=f"vp{tt}_{hf}")
                        for kc in range(8):
                            nc.tensor.matmul(
                                ps[:],
                                lhsT=xT8[:, kc, tt * 128:(tt + 1) * 128],
                                rhs=wv8[:, kc, hf * 512:(hf + 1) * 512],
                                start=(kc == 0), stop=(kc == 7),
                            )
                        dst = Vg[:, tt, hf * 520:(hf + 1) * 520].rearrange(
                            "p (h d) -> p h d", d=65)[:, :, 0:64]
                        # split psum evacuation across ACT and DVE
                        if tt % 2 == 0:
                            nc.scalar.copy(dst, ps[:].rearrange("p (h d) -> p h d", d=64))
                        else:
                            nc.vector.tensor_copy(dst, ps[:].rearrange("p (h d) -> p h d", d=64))

                with tc.tile_pool(name="opsum", bufs=2, space="PSUM") as opsum:
                    # prime: pair-0 scores, then V interleaved with pair-0 AV
                    for qb in range(4):
                        emit_scores_qb(0, qb)
                    emit_v(0)
                    emit_v(1)
                    emit_v(2)
                    emit_av_qb(0, 0)
                    emit_v(3)
                    emit_scores_qb(1, 0)
                    emit_av_qb(0, 1)
                    emit_v(4)
                    emit_scores_qb(1, 1)
                    emit_av_qb(0, 2)
                    emit_v(5)
                    emit_scores_qb(1, 2)
                    emit_av_qb(0, 3)
                    emit_scores_qb(1, 3)
                    emit_evac(0)
                    for p in range(2, 10):
                        for qb in range(4):
                            if p < 8:
                                emit_scores_qb(p, qb)
                            if p <= 8:
                                emit_av_qb(p - 1, qb)
                        if p <= 8:
                            emit_evac(p - 1)
                        if p >= 2:
                            emit_norm(p - 2)

            # ---- Phase E: out = AO.T @ wo + bo_eff ----
            with (
                tc.tile_pool(name="fpsum", bufs=5, space="PSUM") as fpsum,
                tc.tile_pool(name="oout", bufs=3) as oout,
            ):
                for tt in range(4):
                    ot = oout.tile([128, D], F32, tag="oo")
                    for hf in range(2):
                        ps = fpsum.tile([128, 512], F32, tag="fp")
                        nc.tensor.matmul(
                            ps[:], lhsT=onesr[:],
                            rhs=bob16[:, hf * 512:(hf + 1) * 512],
                            start=True, stop=False,
                        )
                        for kc in range(8):
                            nc.tensor.matmul(
                                ps[:],
                                lhsT=AO[kc][:, tt * 128:(tt + 1) * 128],
                                rhs=wo8[:, kc, hf * 512:(hf + 1) * 512],
                                start=False, stop=(kc == 7),
                            )
                        if hf == 0:
                            nc.scalar.copy(ot[:, hf * 512:(hf + 1) * 512], ps[:])
                        else:
                            nc.vector.tensor_copy(ot[:, hf * 512:(hf + 1) * 512], ps[:])
                        nc.sync.dma_start(
                            out=out_d[tt * 128:(tt + 1) * 128, hf * 512:(hf + 1) * 512],
                            in_=ot[:, hf * 512:(hf + 1) * 512])

    nc.compile()
    return nc


def kernel(x, wq, bq, wk, bk, wv, bv, wo, bo):
    bf = ml_dtypes.bfloat16
    x = np.asarray(x, np.float32)
    wq16 = np.ascontiguousarray(np.asarray(wq, np.float32).astype(bf))
    wk16 = np.ascontiguousarray(np.asarray(wk, np.float32).astype(bf))
    wv16 = np.ascontiguousarray(np.asarray(wv, np.float32).astype(bf))
    wo32 = np.asarray(wo, np.float32)
    wo16 = np.ascontiguousarray(wo32.astype(bf))
    bq8 = np.ascontiguousarray(np.asarray(bq, np.float32) * 0.125)
    bk = np.ascontiguousarray(np.asarray(bk, np.float32))
    # fold bv through wo (softmax rows sum to 1): out += bv @ wo + bo
    bo_eff = (np.asarray(bv, np.float32) @ wo32 + np.asarray(bo, np.float32)).astype(np.float32)
    bob16 = np.ascontiguousarray(bo_eff.reshape(1, D).astype(bf))
    onesr = np.ones((1, 128), bf)

    if "nc" not in _cache:
        _cache["nc"] = build_nc()
        # triangle masks: block order [far_h0|far_h1|diag_h0|diag_h1];
        # far valid iff q_idx <= k_idx (q-k = j-i+256 <= 256), diag valid
        # iff q_idx >= k_idx (q-k = j-i >= 0); both include the diagonal
        i = np.arange(128)[:, None]
        j = np.arange(128)[None, :]
        triL = (j <= i).astype(np.float32)
        triU = (j >= i).astype(np.float32)
        mall = np.concatenate([triL, triL, triU, triU], axis=1)
        _cache["mall"] = np.ascontiguousarray(mall.astype(bf))
        # per-core V valid flags: [128, 6, 16]; zero where the halo token is
        # zero-padding (core c==0: tokens 0..255 i.e. chunks 0,1)
        v_ok = np.ones((128, 6, 16), np.float32)
        v_pad = v_ok.copy()
        v_pad[:, 0:2, :] = 0.0
        _cache["valid_ok"] = np.ascontiguousarray(v_ok.astype(bf))
        _cache["valid_pad"] = np.ascontiguousarray(v_pad.astype(bf))
    nc = _cache["nc"]

    in_maps = []
    for core in range(8):
        b, c = divmod(core, 4)
        start = c * TOK
        xh = np.zeros((XT, D), np.float32)
        lo = max(0, start - HALO)
        xh[HALO - (start - lo):] = x[b, lo:start + TOK]
        xhT = np.ascontiguousarray(xh.T.astype(bf))
        in_maps.append({
            "xhT": xhT,
            "wq": wq16, "wk": wk16, "wv": wv16, "wo": wo16,
            "bq": bq8, "bk": bk,
            "valid6": _cache["valid_pad"] if c == 0 else _cache["valid_ok"],
            "mall": _cache["mall"],
            "bob16": bob16, "onesr": onesr,
            "ones64f": np.ones((1, 64), np.float32),
        })
    _cache["last_in_maps"] = in_maps
    res = run_bass_kernel_spmd(nc, in_maps, list(range(8)))
    out = np.empty((B, S, D), np.float32)
    for core in range(8):
        b, c = divmod(core, 4)
        out[b, c * TOK:(c + 1) * TOK] = res.results[core]["out"]
    return out
